# revision 1
# baseline (speedup 1.0000x reference)
"""CaMoE system kernel for 8 Trainium2 NeuronCores.

Sharding: token-parallel (256 tokens/core) for embedding / layernorm /
time-mix / routing; H-sharded dense experts (each core computes a 1/8
hidden-slice of every expert for all tokens); vocab-split head (6283
columns/core). Cross-core traffic via ncfw AllGather / ReduceScatter.
Activations/weights ride the float32r matmul path; the head runs bf16.
"""
import os
import sys
sys.path.insert(0, "/opt/trn_rl_repo")

import numpy as np
import ml_dtypes

import concourse.bass as bass
import concourse.bacc as bacc
import concourse.mybir as mybir
import concourse.tile as tile
from concourse import bass_utils
from concourse.masks import make_identity

F32 = mybir.dt.float32
# float32r (1 cyc/row) flips top-2 routing on knife-edge tokens (min gap 3.7e-5
# vs ~1.6e-4 f32r noise); plain fp32 matches the reference routing exactly.
F32R = mybir.dt.float32
BF16 = mybir.dt.bfloat16
AF = mybir.ActivationFunctionType
OP = mybir.AluOpType
AX = mybir.AxisListType

B, T, C, V, L = 2, 1024, 768, 50257, 2
E_R, E_T, E = 6, 2, 8
H, H2 = 4 * C, 2 * C
EPS = 1e-5
NC_ = 8
P = 128
CT = C // P            # 6 channel k-tiles
TOK = (B * T) // NC_   # 256 tokens per core
M = B * T              # 2048
CH = 512               # token chunk in expert/head phase
NCH = M // CH          # 4
HS = H // NC_          # 384
H2S = H2 // NC_        # 192
RWKV_MT = E_R * HS // P   # 18
TR_MT = E_T * H2S // P    # 3
VS = 6283
VSP = 6400
HEAD_MT = VSP // P        # 50
ECH = TOK                 # expert-phase chunk = one rank's tokens

_CACHE = {}


def _build():
    nc = bacc.Bacc("TRN2", target_bir_lowering=False, debug=False, num_devices=NC_)

    def din(name, shape, dt=F32R):
        return nc.dram_tensor(name, list(shape), dt, kind="ExternalInput")

    x0T = din("x0T", [CT, P, TOK])
    wr = din("wr", [L, C, C]); wk = din("wk", [L, C, C])
    wv = din("wv", [L, C, C]); wo = din("wo", [L, C, C])
    dsig = din("dsig", [L, P, CT], F32)
    scanP = din("scanP", [L, P, CT, NC_], F32)
    ln1s = din("ln1s", [L, P, CT], F32); ln1b = din("ln1b", [L, P, CT], F32)
    ln2s = din("ln2s", [L, P, CT], F32); ln2b = din("ln2b", [L, P, CT], F32)
    lnos = din("lnos", [P, CT], F32); lnob = din("lnob", [P, CT], F32)
    routerW = din("routerW", [L, C, 16])
    confb = din("confb", [L, P, E], F32)
    sharesb = din("sharesb", [L, P, E], F32)
    bridgeW = din("bridgeW", [2 * C, C])
    w1s = din("w1s", [L, C, E_R * HS])
    w2s = din("w2s", [L, E_R * HS, C])
    aps_ = din("aps", [L, C, 2 * E_T * H2S])
    bs_ = din("bs", [L, E_T * H2S, C])
    headw = din("headw", [C, VSP], BF16)
    sel8c = din("sel8c", [E, E, P])
    ind8c = din("ind8c", [E, TR_MT, P])
    onescol = din("onescol", [P, 1])
    onesrow = din("onesrow", [1, P])

    logitsT = nc.dram_tensor("logitsT", [VSP, M], BF16, kind="ExternalOutput")

    fin_in = nc.dram_tensor("fin_in", [P, CT], F32)
    fin_out = nc.dram_tensor("fin_out", [NC_ * P, CT], F32, addr_space="Shared")
    bids_in = nc.dram_tensor("bids_in", [TOK, E], F32)
    bids_out = nc.dram_tensor("bids_out", [M, E], F32, addr_space="Shared")
    hp_in = nc.dram_tensor("hp_in", [2 * CT, P, TOK], F32R)
    hp_out = nc.dram_tensor("hp_out", [NC_, 2 * CT, P, TOK], F32R,
                            addr_space="Shared")
    rs_in = nc.dram_tensor("rs_in", [NC_, CT, P, TOK], F32)
    rs_out = nc.dram_tensor("rs_out", [CT, P, TOK], F32)
    xf_in = nc.dram_tensor("xf_in", [CT, P, TOK], F32)
    xf_out = nc.dram_tensor("xf_out", [NC_, CT, P, TOK], F32, addr_space="Shared")

    RG = [list(range(NC_))]

    ctxs = []

    tc = tile.TileContext(nc)
    tc.__enter__()
    try:
        def pool(name, bufs, space="SBUF"):
            p_ = tc.tile_pool(name=name, bufs=bufs, space=space)
            ctxs.append(p_)
            return p_.__enter__()

        cpool = pool("const", 1)
        xpool = pool("xp", 1)
        wkp = pool("wk", 1)
        hfp = pool("hf", 1)
        h1p = pool("h1", 1)
        wst = pool("wst", 3)
        stg = pool("stg", 3)
        psp = pool("ps", 5, space="PSUM")
        pss = pool("pss", 1, space="PSUM")

        ident = cpool.tile([P, P], F32)
        make_identity(nc, ident[:])
        ones_col = cpool.tile([P, 1], F32R)
        nc.sync.dma_start(out=ones_col[:], in_=onescol.ap())
        ones_row_r = cpool.tile([1, P], F32R)
        nc.sync.dma_start(out=ones_row_r[:], in_=onesrow.ap())
        # expert-row selectors: sel[:, e, :] is a [E, P] lhsT picking G row e
        sel8 = cpool.tile([E, E, P], F32R)
        nc.sync.dma_start(out=sel8[:], in_=sel8c.ap())
        ind8 = cpool.tile([E, TR_MT, P], F32R)
        nc.sync.dma_start(out=ind8[:], in_=ind8c.ap())

        def c32(ap):
            return ap.bitcast(F32)

        x = xpool.tile([P, CT, TOK], F32R, tag="x")
        nc.sync.dma_start(out=x[:], in_=x0T.ap().rearrange("kt p t -> p kt t"))
        vf = xpool.tile([P, CT, TOK], F32R, tag="vf")

        def layer_norm(src, s_ap, b_ap):
            """src [P, CT, TOK] f32r -> normalized f32r tile (tag lnout)."""
            x2 = wkp.tile([P, CT, TOK], F32R, tag="lnx2")
            nc.vector.tensor_tensor(out=x2[:], in0=src[:], in1=src[:], op=OP.mult)
            ps_st = pss.tile([1, 2, TOK], F32, space="PSUM", tag="lnstats")
            for k in range(CT):
                nc.tensor.matmul(ps_st[:, 0, :], c32(ones_col[:]), c32(src[:, k, :]),
                                 start=(k == 0), stop=(k == CT - 1))
            for k in range(CT):
                nc.tensor.matmul(ps_st[:, 1, :], c32(ones_col[:]), c32(x2[:, k, :]),
                                 start=(k == 0), stop=(k == CT - 1))
            stats = wkp.tile([1, 4, TOK], F32, tag="lnsts")
            mean, ex2, m2, var = (stats[:, i, :] for i in range(4))
            nc.vector.tensor_scalar(out=mean, in0=ps_st[:, 0, :], scalar1=1.0 / C,
                                    scalar2=None, op0=OP.mult)
            nc.vector.tensor_scalar(out=ex2, in0=ps_st[:, 1, :], scalar1=1.0 / C,
                                    scalar2=None, op0=OP.mult)
            nc.vector.tensor_tensor(out=m2, in0=mean, in1=mean, op=OP.mult)
            nc.vector.tensor_tensor(out=var, in0=ex2, in1=m2, op=OP.subtract)
            nc.vector.tensor_scalar(out=var, in0=var, scalar1=EPS, scalar2=None,
                                    op0=OP.add)
            nr = wkp.tile([1, 2, TOK], F32R, tag="lnnr")
            rstd_f = wkp.tile([1, TOK], F32, tag="lnrstd")
            nc.scalar.activation(rstd_f[:], var, AF.Sqrt)
            nc.vector.reciprocal(rstd_f[:], rstd_f[:])
            nc.vector.tensor_copy(nr[:, 0, :], rstd_f[:])
            nc.vector.tensor_scalar(out=nr[:, 1, :], in0=mean, scalar1=-1.0,
                                    scalar2=None, op0=OP.mult)
            ps_b = pss.tile([P, 2, TOK], F32, space="PSUM", tag="lnb")
            nc.tensor.matmul(ps_b[:, 0, :], c32(ones_row_r[:]), c32(nr[:, 0, :]),
                             start=True, stop=True)
            nc.tensor.matmul(ps_b[:, 1, :], c32(ones_row_r[:]), c32(nr[:, 1, :]),
                             start=True, stop=True)
            out = wkp.tile([P, CT, TOK], F32R, tag="lnout")
            for k in range(CT):
                nc.vector.tensor_tensor(out=out[:, k, :], in0=src[:, k, :],
                                        in1=ps_b[:, 1, :], op=OP.add)
                nc.vector.tensor_tensor(out=out[:, k, :], in0=out[:, k, :],
                                        in1=ps_b[:, 0, :], op=OP.mult)
            st = wkp.tile([P, 2, CT], F32, tag="lnsc")
            nc.sync.dma_start(out=st[:, 0, :], in_=s_ap)
            nc.sync.dma_start(out=st[:, 1, :], in_=b_ap)
            for k in range(CT):
                nc.vector.tensor_scalar(out=out[:, k, :], in0=out[:, k, :],
                                        scalar1=st[:, 0, k:k + 1],
                                        scalar2=st[:, 1, k:k + 1],
                                        op0=OP.mult, op1=OP.add)
            return out

        def proj(w_dram_l, rhs, out, act=None, accum_into=None):
            for m in range(CT):
                w_t = wst.tile([P, CT, P], F32R, tag="wst6")
                nc.sync.dma_start(
                    out=w_t[:],
                    in_=w_dram_l[:, m * P:(m + 1) * P].rearrange(
                        "(kt p) n -> p kt n", p=P))
                ps = psp.tile([P, CH], F32, space="PSUM", tag="mm")
                for k in range(CT):
                    nc.tensor.matmul(ps[:, 0:TOK], c32(w_t[:, k, :]),
                                     c32(rhs[:, k, :]),
                                     start=(k == 0), stop=(k == CT - 1))
                if accum_into is not None:
                    nc.vector.tensor_tensor(out=accum_into[:, m, :],
                                            in0=accum_into[:, m, :],
                                            in1=ps[:, 0:TOK], op=OP.add)
                elif act is not None:
                    nc.scalar.activation(out[:, m, :], ps[:, 0:TOK], act)
                else:
                    nc.vector.tensor_copy(out[:, m, :], ps[:, 0:TOK])

        for l in range(L):
            xln = layer_norm(x, ln1s.ap()[l], ln1b.ap()[l])

            sigr = wkp.tile([P, CT, TOK], F32, tag="sigr")
            kk = wkp.tile([P, CT, TOK], F32, tag="kk")
            vv = wkp.tile([P, CT, TOK], F32, tag="vv")
            proj(wr.ap()[l], xln, sigr, act=AF.Sigmoid)
            proj(wk.ap()[l], xln, kk)
            proj(wv.ap()[l], xln, vv)
            if l == 0:
                nc.vector.tensor_copy(vf[:], vv[:])
            else:
                nc.vector.tensor_tensor(out=vv[:], in0=vv[:], in1=vf[:], op=OP.add)
                nc.vector.tensor_scalar(out=vv[:], in0=vv[:], scalar1=0.5,
                                        scalar2=None, op0=OP.mult)
            kv = wkp.tile([P, CT, TOK], F32, tag="kv")
            nc.vector.tensor_tensor(out=kv[:], in0=kk[:], in1=vv[:], op=OP.mult)

            dtile = wkp.tile([P, CT], F32, tag="dt")
            nc.sync.dma_start(out=dtile[:], in_=dsig.ap()[l])
            states = wkp.tile([P, CT, TOK], F32R, tag="states")
            for k in range(CT):
                nc.vector.tensor_tensor_scan(
                    states[:, k, :], dtile[:, k:k + 1].to_broadcast([P, TOK]),
                    kv[:, k, :], 0.0, op0=OP.mult, op1=OP.add)
            fin = wkp.tile([P, CT], F32, tag="fin")
            for k in range(CT):
                nc.vector.tensor_copy(fin[:, k:k + 1], states[:, k, TOK - 1:TOK])
            nc.sync.dma_start(out=fin_in.ap(), in_=fin[:])
            nc.gpsimd.collective_compute(
                "AllGather", OP.bypass, replica_groups=RG,
                ins=[fin_in.ap().opt()], outs=[fin_out.ap().opt()])
            lt = wkp.tile([P, CT, NC_], F32, tag="lfin")
            nc.sync.dma_start(out=lt[:],
                              in_=fin_out.ap().rearrange("(m p) kt -> p kt m", p=P))
            pt = wkp.tile([P, CT, NC_], F32, tag="pfin")
            nc.sync.dma_start(out=pt[:], in_=scanP.ap()[l])
            nc.vector.tensor_tensor(out=lt[:], in0=lt[:], in1=pt[:], op=OP.mult)
            init = wkp.tile([P, CT], F32, tag="init")
            nc.vector.tensor_reduce(init[:], lt[:], axis=AX.X, op=OP.add)
            for k in range(CT):
                nc.vector.tensor_tensor_scan(
                    states[:, k, :], dtile[:, k:k + 1].to_broadcast([P, TOK]),
                    kv[:, k, :], init[:, k:k + 1], op0=OP.mult, op1=OP.add)
            satt = wkp.tile([P, CT, TOK], F32R, tag="lnx2")
            nc.vector.tensor_tensor(out=satt[:], in0=sigr[:], in1=states[:],
                                    op=OP.mult)
            proj(wo.ap()[l], satt, None, accum_into=x)

            h = layer_norm(x, ln2s.ap()[l], ln2b.ap()[l])

            # router
            rwt = wkp.tile([P, CT, 16], F32R, tag="rwt")
            nc.sync.dma_start(out=rwt[:],
                              in_=routerW.ap()[l].rearrange("(kt p) n -> p kt n", p=P))
            se_t = wkp.tile([P, 2, E], F32, tag="sht")
            nc.sync.dma_start(out=se_t[:, 0, :], in_=sharesb.ap()[l])
            nc.sync.dma_start(out=se_t[:, 1, :], in_=confb.ap()[l])
            bids_sb = wkp.tile([P, TOK // P, E], F32, tag="bids")
            for m in range(TOK // P):
                ps = psp.tile([P, CH], F32, space="PSUM", tag="mm")
                for k in range(CT):
                    nc.tensor.matmul(ps[:, 0:16], c32(h[:, k, m * P:(m + 1) * P]),
                                     c32(rwt[:, k, :]), start=(k == 0),
                                     stop=(k == CT - 1))
                tmp = wkp.tile([P, E], F32, tag="rtmp")
                nc.vector.tensor_tensor(out=tmp[:], in0=ps[:, 0:E],
                                        in1=se_t[:, 1, :], op=OP.add)
                nc.scalar.activation(tmp[:], tmp[:], AF.Sigmoid)
                nc.vector.tensor_tensor(out=tmp[:], in0=tmp[:], in1=se_t[:, 0, :],
                                        op=OP.mult)
                nc.vector.tensor_tensor(out=bids_sb[:, m, :], in0=tmp[:],
                                        in1=ps[:, E:16], op=OP.add)
            nc.sync.dma_start(out=bids_in.ap().rearrange("(m p) e -> p m e", p=P),
                              in_=bids_sb[:])
            nc.gpsimd.collective_compute(
                "AllGather", OP.bypass, replica_groups=RG,
                ins=[bids_in.ap().opt()], outs=[bids_out.ap().opt()])

            # bridge -> prefix
            prefix = wkp.tile([P, CT, TOK], F32R, tag="prefix")
            for m in range(CT):
                w_t = wst.tile([P, 2 * CT, P], F32R, tag="wst12")
                nc.sync.dma_start(
                    out=w_t[:],
                    in_=bridgeW.ap()[:, m * P:(m + 1) * P].rearrange(
                        "(kt p) n -> p kt n", p=P))
                ps = psp.tile([P, CH], F32, space="PSUM", tag="mm")
                for k in range(2 * CT):
                    rhs = h[:, k, :] if k < CT else states[:, k - CT, :]
                    if l == 0:
                        nc.tensor.matmul(ps[:, 0:TOK], c32(w_t[:, k, :]), c32(rhs),
                                         start=(k == 0), stop=(k == 2 * CT - 1))
                    else:
                        nc.tensor.matmul(ps[:, 0:TOK], w_t[:, k, :], rhs,
                                         start=(k == 0), stop=(k == 2 * CT - 1))
                nc.scalar.activation(prefix[:, m, :], ps[:, 0:TOK], AF.Tanh)

            nc.sync.dma_start(out=hp_in.ap()[0:CT].rearrange("kt p t -> p kt t"),
                              in_=h[:])
            nc.sync.dma_start(
                out=hp_in.ap()[CT:2 * CT].rearrange("kt p t -> p kt t"),
                in_=prefix[:])
            nc.gpsimd.collective_compute(
                "AllGather", OP.bypass, replica_groups=RG,
                ins=[hp_in.ap().opt()], outs=[hp_out.ap().opt()])

            # gates from gathered bids: G [E, 16, P], token t = g*128 + p
            bt = wkp.tile([P, 16, E], F32, tag="btile")
            nc.sync.dma_start(out=bt[:],
                              in_=bids_out.ap().rearrange("(g p) e -> p g e", p=P))
            m1 = wkp.tile([P, 2, 16], F32, tag="m1")
            nc.vector.tensor_reduce(m1[:, 0, :], bt[:], axis=AX.X, op=OP.max)
            eq1 = wkp.tile([P, 16, E], F32, tag="eq1")
            nc.vector.tensor_tensor(out=eq1[:], in0=bt[:],
                                    in1=m1[:, 0, :].to_broadcast([P, 16, E]),
                                    op=OP.is_equal)
            msk = wkp.tile([P, 16, E], F32, tag="msk")
            nc.vector.scalar_tensor_tensor(out=msk[:], in0=eq1[:], scalar=-1e30,
                                           in1=bt[:], op0=OP.mult, op1=OP.add)
            nc.vector.tensor_reduce(m1[:, 1, :], msk[:], axis=AX.X, op=OP.max)
            eq2 = wkp.tile([P, 16, E], F32, tag="eq2")
            nc.vector.tensor_tensor(out=eq2[:], in0=msk[:],
                                    in1=m1[:, 1, :].to_broadcast([P, 16, E]),
                                    op=OP.is_equal)
            wg = wkp.tile([P, 2, 16], F32, tag="wg")
            nc.vector.tensor_tensor(out=wg[:, 1, :], in0=m1[:, 1, :],
                                    in1=m1[:, 0, :], op=OP.subtract)
            nc.scalar.activation(wg[:, 1, :], wg[:, 1, :], AF.Sigmoid)
            nc.vector.tensor_scalar(out=wg[:, 0, :], in0=wg[:, 1, :], scalar1=-1.0,
                                    scalar2=1.0, op0=OP.mult, op1=OP.add)
            gt = wkp.tile([P, 16, E], F32, tag="gt")
            nc.vector.tensor_tensor(out=gt[:], in0=eq1[:],
                                    in1=wg[:, 0, :].to_broadcast([P, 16, E]),
                                    op=OP.mult)
            g2t = wkp.tile([P, 16, E], F32, tag="g2t")
            nc.vector.tensor_tensor(out=g2t[:], in0=eq2[:],
                                    in1=wg[:, 1, :].to_broadcast([P, 16, E]),
                                    op=OP.mult)
            nc.vector.tensor_tensor(out=gt[:], in0=gt[:], in1=g2t[:], op=OP.add)
            G = wkp.tile([E, 16, P], F32R, tag="G")
            for g in range(16):
                psg = pss.tile([E, P], F32, space="PSUM", tag="psg")
                nc.tensor.transpose(psg[:], gt[:, g, :], ident[:])
                nc.vector.tensor_copy(G[:, g, :], psg[:])

            # ---- expert phase (dense, H-sharded); chunk = one rank's tokens
            for c in range(NC_):
                h_ch = h1p.tile([P, CT, ECH], F32R, tag="hch")
                pf_ch = h1p.tile([P, CT, ECH], F32R, tag="pfch")
                nc.sync.dma_start(
                    out=h_ch[:],
                    in_=hp_out.ap()[c, 0:CT].rearrange("kt p t -> p kt t"))
                nc.sync.dma_start(
                    out=pf_ch[:],
                    in_=hp_out.ap()[c, CT:2 * CT].rearrange("kt p t -> p kt t"))
                h1g = h1p.tile([P, RWKV_MT + TR_MT, ECH], F32R, tag="h1g")
                for mt in range(RWKV_MT):
                    e = mt // (HS // P)
                    w_t = wst.tile([P, CT, P], F32R, tag="wst6")
                    nc.sync.dma_start(
                        out=w_t[:],
                        in_=w1s.ap()[l][:, mt * P:(mt + 1) * P].rearrange(
                            "(kt p) n -> p kt n", p=P))
                    ps = psp.tile([P, CH], F32, space="PSUM", tag="mm")
                    for k in range(CT):
                        if l == 0:
                            nc.tensor.matmul(ps[:, 0:ECH], c32(w_t[:, k, :]),
                                             c32(h_ch[:, k, :]),
                                             start=(k == 0), stop=(k == CT - 1))
                        else:
                            nc.tensor.matmul(ps[:, 0:ECH], w_t[:, k, :],
                                             h_ch[:, k, :],
                                             start=(k == 0), stop=(k == CT - 1))
                    rl = stg.tile([P, ECH], F32, tag="rl")
                    nc.scalar.activation(rl[:], ps[:, 0:ECH], AF.Relu)
                    nc.scalar.activation(h1g[:, mt, :], rl[:], AF.Square)
                    gps = psp.tile([P, CH], F32, space="PSUM", tag="mm")
                    gsl = G[:, 2 * c:2 * (c + 1), :].rearrange("a g p -> a (g p)")
                    if l == 0:
                        nc.tensor.matmul(gps[:, 0:ECH], c32(sel8[:, e, :]), c32(gsl),
                                         start=True, stop=True)
                    else:
                        nc.tensor.matmul(gps[:, 0:ECH], sel8[:, e, :], gsl,
                                         start=True, stop=True)
                    nc.vector.tensor_tensor(out=h1g[:, mt, :], in0=h1g[:, mt, :],
                                            in1=gps[:, 0:ECH], op=OP.mult)
                # trans experts
                pz_t = h1p.tile([P, TR_MT, ECH], F32, tag="pz_t")
                for mt in range(TR_MT):
                    w_t = wst.tile([P, CT, P], F32R, tag="wst6")
                    nc.sync.dma_start(
                        out=w_t[:],
                        in_=aps_.ap()[l][:, (TR_MT + mt) * P:(TR_MT + mt + 1) * P]
                        .rearrange("(kt p) n -> p kt n", p=P))
                    ps = psp.tile([P, CH], F32, space="PSUM", tag="mm")
                    for k in range(CT):
                        if l == 0:
                            nc.tensor.matmul(ps[:, 0:ECH], c32(w_t[:, k, :]),
                                             c32(pf_ch[:, k, :]),
                                             start=(k == 0), stop=(k == CT - 1))
                        else:
                            nc.tensor.matmul(ps[:, 0:ECH], w_t[:, k, :],
                                             pf_ch[:, k, :],
                                             start=(k == 0), stop=(k == CT - 1))
                    nc.vector.tensor_copy(pz_t[:, mt, :], ps[:, 0:ECH])
                for mt in range(TR_MT):
                    w_t = wst.tile([P, CT, P], F32R, tag="wst6")
                    nc.sync.dma_start(
                        out=w_t[:],
                        in_=aps_.ap()[l][:, mt * P:(mt + 1) * P].rearrange(
                            "(kt p) n -> p kt n", p=P))
                    ps = psp.tile([P, CH], F32, space="PSUM", tag="mm")
                    for k in range(CT):
                        if l == 0:
                            nc.tensor.matmul(ps[:, 0:ECH], c32(w_t[:, k, :]),
                                             c32(h_ch[:, k, :]),
                                             start=(k == 0), stop=(k == CT - 1))
                        else:
                            nc.tensor.matmul(ps[:, 0:ECH], w_t[:, k, :],
                                             h_ch[:, k, :],
                                             start=(k == 0), stop=(k == CT - 1))
                    aidx = RWKV_MT + mt
                    nc.scalar.activation(h1g[:, aidx, :], ps[:, 0:ECH], AF.Silu)
                    nc.vector.tensor_tensor(out=h1g[:, aidx, :],
                                            in0=h1g[:, aidx, :],
                                            in1=pz_t[:, mt, :], op=OP.mult)
                    gps = psp.tile([P, CH], F32, space="PSUM", tag="mm")
                    gsl = G[:, 2 * c:2 * (c + 1), :].rearrange("a g p -> a (g p)")
                    if l == 0:
                        nc.tensor.matmul(gps[:, 0:ECH], c32(ind8[:, mt, :]), c32(gsl),
                                         start=True, stop=True)
                    else:
                        nc.tensor.matmul(gps[:, 0:ECH], ind8[:, mt, :], gsl,
                                         start=True, stop=True)
                    nc.vector.tensor_tensor(out=h1g[:, aidx, :],
                                            in0=h1g[:, aidx, :],
                                            in1=gps[:, 0:ECH], op=OP.mult)
                # second matmuls accumulate all experts
                for m in range(CT):
                    ps = psp.tile([P, CH], F32, space="PSUM", tag="mm")
                    for e in range(E_R):
                        w_t = wst.tile([P, HS // P, P], F32R, tag="wst3")
                        nc.sync.dma_start(
                            out=w_t[:],
                            in_=w2s.ap()[l][e * HS:(e + 1) * HS, m * P:(m + 1) * P]
                            .rearrange("(kt p) n -> p kt n", p=P))
                        for k2 in range(HS // P):
                            if l == 0:
                                nc.tensor.matmul(ps[:, 0:ECH], c32(w_t[:, k2, :]),
                                                 c32(h1g[:, e * (HS // P) + k2, :]),
                                                 start=(e == 0 and k2 == 0),
                                                 stop=False)
                            else:
                                nc.tensor.matmul(ps[:, 0:ECH], w_t[:, k2, :],
                                                 h1g[:, e * (HS // P) + k2, :],
                                                 start=(e == 0 and k2 == 0),
                                                 stop=False)
                    w_t = wst.tile([P, TR_MT, P], F32R, tag="wst3")
                    nc.sync.dma_start(
                        out=w_t[:],
                        in_=bs_.ap()[l][:, m * P:(m + 1) * P].rearrange(
                            "(kt p) n -> p kt n", p=P))
                    for k2 in range(TR_MT):
                        if l == 0:
                            nc.tensor.matmul(ps[:, 0:ECH], c32(w_t[:, k2, :]),
                                             c32(h1g[:, RWKV_MT + k2, :]),
                                             start=False, stop=(k2 == TR_MT - 1))
                        else:
                            nc.tensor.matmul(ps[:, 0:ECH], w_t[:, k2, :],
                                             h1g[:, RWKV_MT + k2, :],
                                             start=False, stop=(k2 == TR_MT - 1))
                    st = stg.tile([P, ECH], F32, tag="w2out")
                    if m % 2 == 0:
                        nc.vector.tensor_copy(st[:], ps[:, 0:ECH])
                    else:
                        nc.scalar.activation(st[:], ps[:, 0:ECH], AF.Copy)
                    nc.sync.dma_start(out=rs_in.ap()[c, m], in_=st[:])
            nc.gpsimd.collective_compute(
                "ReduceScatter", OP.add, replica_groups=RG,
                ins=[rs_in.ap().opt()], outs=[rs_out.ap().opt()])
            moe = wkp.tile([P, CT, TOK], F32, tag="kv")
            nc.sync.dma_start(out=moe[:],
                              in_=rs_out.ap().rearrange("kt p t -> p kt t"))
            nc.vector.tensor_tensor(out=x[:], in0=x[:], in1=moe[:], op=OP.add)

        # final layernorm + allgather + head
        xf = layer_norm(x, lnos.ap(), lnob.ap())
        xfc = wkp.tile([P, CT, TOK], F32, tag="kv")
        nc.vector.tensor_copy(xfc[:], xf[:])
        nc.sync.dma_start(out=xf_in.ap().rearrange("kt p t -> p kt t"), in_=xfc[:])
        nc.gpsimd.collective_compute(
            "AllGather", OP.bypass, replica_groups=RG,
            ins=[xf_in.ap().opt()], outs=[xf_out.ap().opt()])
        xf_full = hfp.tile([P, CT, M], BF16, tag="xffull")
        for r in range(NC_):
            nc.gpsimd.dma_start(
                out=xf_full[:, :, r * TOK:(r + 1) * TOK],
                in_=xf_out.ap()[r].rearrange("kt p t -> p kt t"))
        for m in range(HEAD_MT):
            w_t = wst.tile([P, CT, P], BF16, tag="whead")
            nc.sync.dma_start(
                out=w_t[:],
                in_=headw.ap()[:, m * P:(m + 1) * P].rearrange(
                    "(kt p) n -> p kt n", p=P))
            for c4 in range(NCH):
                ps = psp.tile([P, CH], F32, space="PSUM", tag="mm")
                for k in range(CT):
                    nc.tensor.matmul(ps[:], w_t[:, k, :],
                                     xf_full[:, k, c4 * CH:(c4 + 1) * CH],
                                     start=(k == 0), stop=(k == CT - 1))
                st = stg.tile([P, CH], BF16, tag="hout")
                if c4 % 2 == 0:
                    nc.vector.tensor_copy(st[:], ps[:])
                else:
                    nc.scalar.activation(st[:], ps[:], AF.Copy)
                nc.sync.dma_start(
                    out=logitsT.ap()[m * P:(m + 1) * P, c4 * CH:(c4 + 1) * CH],
                    in_=st[:])
    finally:
        for p_ in reversed(ctxs):
            p_.__exit__(None, None, None)
        tc.__exit__(None, None, None)

    nc.compile()
    return nc


def _sel8_const():
    s = np.zeros((E, E, P), np.float32)
    for e in range(E):
        s[e, e, :] = 1.0
    return s


def _ind8_const():
    s = np.zeros((E, TR_MT, P), np.float32)
    s[E_R, 0, :] = 1.0
    s[E_R, 1, 0:64] = 1.0
    s[E_R + 1, 1, 64:128] = 1.0
    s[E_R + 1, 2, :] = 1.0
    return s


def _host_prep(inputs):
    f32 = np.float32
    idx = np.asarray(inputs["idx"]).astype(np.int64)
    emb_W = np.asarray(inputs["emb_W"], dtype=f32)
    x0 = emb_W[idx.reshape(-1)]                      # [M, C]
    decay = np.asarray(inputs["decay"], dtype=f32)
    d = (1.0 / (1.0 + np.exp(-decay.astype(np.float64)))).astype(np.float64)  # [L,C]
    caps = np.asarray(inputs["capital_shares"], dtype=f32)
    shares = caps / caps.sum(axis=1, keepdims=True)  # [L, E]

    def chanlay(a):
        a = np.asarray(a, dtype=f32)
        return np.ascontiguousarray(a.reshape(*a.shape[:-1], CT, P).swapaxes(-1, -2))

    conf_w = np.asarray(inputs["conf_w"], dtype=f32)
    critic = np.asarray(inputs["critic_Wa"], dtype=f32)
    routerW = np.ascontiguousarray(
        np.concatenate([conf_w.transpose(0, 2, 1), critic], axis=2))

    ffn_W1 = np.asarray(inputs["ffn_W1"], dtype=f32)
    ffn_W2 = np.asarray(inputs["ffn_W2"], dtype=f32)
    tA = np.asarray(inputs["trans_A"], dtype=f32)
    tP = np.asarray(inputs["trans_P"], dtype=f32)
    tB = np.asarray(inputs["trans_B"], dtype=f32)
    head_W = np.asarray(inputs["head_W"], dtype=f32)
    conf_b = np.asarray(inputs["conf_b"], dtype=f32)  # [L, E]

    shared = dict(
        wr=np.ascontiguousarray(inputs["Wr"], dtype=f32),
        wk=np.ascontiguousarray(inputs["Wk"], dtype=f32),
        wv=np.ascontiguousarray(inputs["Wv"], dtype=f32),
        wo=np.ascontiguousarray(inputs["Wo"], dtype=f32),
        dsig=chanlay(d.astype(f32)),
        ln1s=chanlay(inputs["ln1_s"]), ln1b=chanlay(inputs["ln1_b"]),
        ln2s=chanlay(inputs["ln2_s"]), ln2b=chanlay(inputs["ln2_b"]),
        lnos=chanlay(inputs["lnout_s"]), lnob=chanlay(inputs["lnout_b"]),
        routerW=routerW,
        confb=np.ascontiguousarray(
            np.broadcast_to(conf_b[:, None, :], (L, P, E)).astype(f32)),
        sharesb=np.ascontiguousarray(
            np.broadcast_to(shares[:, None, :], (L, P, E)).astype(f32)),
        bridgeW=np.ascontiguousarray(inputs["bridge_W"], dtype=f32),
        sel8c=_sel8_const(), ind8c=_ind8_const(),
        onescol=np.ones((P, 1), np.float32),
        onesrow=np.ones((1, P), np.float32),
    )

    in_maps = []
    for i in range(NC_):
        b_idx, j = divmod(i, NC_ // B)
        scanP_l = np.zeros((L, C, NC_), np.float64)
        for ll in range(L):
            for mprev in range(j):
                ridx = b_idx * (NC_ // B) + mprev
                scanP_l[ll, :, ridx] = d[ll] ** (256.0 * (j - mprev - 1))
        scanP_lay = np.ascontiguousarray(
            scanP_l.astype(f32).reshape(L, CT, P, NC_).swapaxes(1, 2))

        w1c = ffn_W1[:, :, :, i * HS:(i + 1) * HS]
        w1c = np.ascontiguousarray(w1c.transpose(0, 2, 1, 3).reshape(L, C, E_R * HS))
        w2c = np.ascontiguousarray(
            ffn_W2[:, :, i * HS:(i + 1) * HS, :].reshape(L, E_R * HS, C))
        a_s = tA[:, :, :, i * H2S:(i + 1) * H2S].transpose(0, 2, 1, 3)
        a_s = a_s.reshape(L, C, E_T * H2S)
        p_s = tP[:, :, :, i * H2S:(i + 1) * H2S].transpose(0, 2, 1, 3)
        p_s = p_s.reshape(L, C, E_T * H2S)
        apsc = np.ascontiguousarray(np.concatenate([a_s, p_s], axis=2))
        b_c = np.ascontiguousarray(
            tB[:, :, i * H2S:(i + 1) * H2S, :].reshape(L, E_T * H2S, C))

        hw = np.zeros((C, VSP), f32)
        lo = i * VS
        hi = min((i + 1) * VS, V)
        hw[:, :hi - lo] = head_W[:, lo:hi]

        x0T = np.ascontiguousarray(
            x0[i * TOK:(i + 1) * TOK].T.reshape(CT, P, TOK))

        im = dict(shared)
        im.update(
            x0T=x0T.astype(f32),
            scanP=scanP_lay,
            w1s=w1c, w2s=w2c, aps=apsc, bs=b_c,
            headw=np.ascontiguousarray(hw.astype(ml_dtypes.bfloat16)),
        )
        in_maps.append(im)
    return in_maps


def kernel(**inputs):
    if "nc" not in _CACHE:
        _CACHE["nc"] = _build()
    nc = _CACHE["nc"]
    in_maps = _host_prep(inputs)
    trace = os.environ.get("K_TRACE", "0") == "1"
    res = bass_utils.run_bass_kernel_spmd(nc, in_maps, core_ids=list(range(NC_)),
                                          trace=trace)
    _CACHE["last_res"] = res
    outs = []
    for i in range(NC_):
        lt = np.asarray(res.results[i]["logitsT"], dtype=np.float32)
        lo = i * VS
        hi = min((i + 1) * VS, V)
        outs.append(lt[: hi - lo].T)
    full = np.concatenate(outs, axis=1)
    return full.reshape(B, T, V).astype(np.float32)



# revision 20
# speedup vs baseline: 1.0172x; 1.0172x over previous
"""CaMoE system kernel for 8 Trainium2 NeuronCores.

Sharding: token-parallel (256 tokens/core) for embedding / layernorm /
time-mix / routing; H-sharded dense experts (each core computes a 1/8
hidden-slice of every expert for all tokens); vocab-split head (6283
columns/core). Cross-core traffic via ncfw AllGather / ReduceScatter.
Activations/weights ride the float32r matmul path; the head runs bf16.
"""
import os
import sys
sys.path.insert(0, "/opt/trn_rl_repo")

import numpy as np
import ml_dtypes

import concourse.bass as bass
import concourse.bacc as bacc
import concourse.mybir as mybir
import concourse.tile as tile
from concourse import bass_utils
from concourse.masks import make_identity

F32 = mybir.dt.float32
# float32r (1 cyc/row) flips top-2 routing on knife-edge tokens (min gap 3.7e-5
# vs ~1.6e-4 f32r noise); plain fp32 matches the reference routing exactly.
F32R = mybir.dt.float32
BF16 = mybir.dt.bfloat16
AF = mybir.ActivationFunctionType
OP = mybir.AluOpType
AX = mybir.AxisListType

B, T, C, V, L = 2, 1024, 768, 50257, 2
E_R, E_T, E = 6, 2, 8
H, H2 = 4 * C, 2 * C
EPS = 1e-5
NC_ = 8
P = 128
CT = C // P            # 6 channel k-tiles
TOK = (B * T) // NC_   # 256 tokens per core
M = B * T              # 2048
CH = 512               # token chunk in expert/head phase
NCH = M // CH          # 4
HS = H // NC_          # 384
H2S = H2 // NC_        # 192
RWKV_MT = E_R * HS // P   # 18
TR_MT = E_T * H2S // P    # 3
VS = 6283
VSP = 6400
HEAD_MT = VSP // P        # 50
ECH = TOK                 # expert-phase chunk = one rank's tokens

_CACHE = {}


def _build():
    nc = bacc.Bacc("TRN2", target_bir_lowering=False, debug=False, num_devices=NC_)

    def din(name, shape, dt=F32R):
        return nc.dram_tensor(name, list(shape), dt, kind="ExternalInput")

    x0T = din("x0T", [CT, P, TOK])
    wr = din("wr", [L, C, C]); wk = din("wk", [L, C, C])
    wv = din("wv", [L, C, C]); wo = din("wo", [L, C, C])
    dsig = din("dsig", [L, P, CT], F32)
    scanP = din("scanP", [L, P, CT, NC_], F32)
    ln1s = din("ln1s", [L, P, CT], F32); ln1b = din("ln1b", [L, P, CT], F32)
    ln2s = din("ln2s", [L, P, CT], F32); ln2b = din("ln2b", [L, P, CT], F32)
    lnos = din("lnos", [P, CT], F32); lnob = din("lnob", [P, CT], F32)
    routerW = din("routerW", [L, C, 16])
    confb = din("confb", [L, P, E], F32)
    sharesb = din("sharesb", [L, P, E], F32)
    bridgeW = din("bridgeW", [2 * C, C])
    bridgeb = din("bridgeb", [2 * C, C], BF16)
    w1s = din("w1s", [L, C, E_R * HS])
    w2s = din("w2s", [L, E_R * HS, C])
    aps_ = din("aps", [L, C, 2 * E_T * H2S])
    bs_ = din("bs", [L, E_T * H2S, C])
    w1b = din("w1b", [C, E_R * HS], BF16)
    w2b = din("w2b", [E_R * HS, C], BF16)
    apb = din("apb", [C, 2 * E_T * H2S], BF16)
    bsb = din("bsb", [E_T * H2S, C], BF16)
    headw = din("headw", [C, VSP], BF16)
    sel8b = din("sel8b", [E, E, P], BF16)
    ind8b = din("ind8b", [E, TR_MT, P], BF16)
    sel8c = din("sel8c", [E, E, P])
    ind8c = din("ind8c", [E, TR_MT, P])
    onescol = din("onescol", [P, 1])
    onesrow = din("onesrow", [1, P])

    logitsT = nc.dram_tensor("logitsT", [VSP, M], BF16, kind="ExternalOutput")

    fin_in = nc.dram_tensor("fin_in", [P, CT], F32)
    fin_out = nc.dram_tensor("fin_out", [NC_ * P, CT], F32, addr_space="Shared")
    bids_in = nc.dram_tensor("bids_in", [TOK, E], F32)
    bids_out = nc.dram_tensor("bids_out", [M, E], F32, addr_space="Shared")
    hp_in = nc.dram_tensor("hp_in", [2 * CT, P, TOK], F32R)
    hp_out = nc.dram_tensor("hp_out", [NC_, 2 * CT, P, TOK], F32R,
                            addr_space="Shared")
    rs_in = nc.dram_tensor("rs_in", [NC_, CT, P, TOK], F32)
    rs_out = nc.dram_tensor("rs_out", [CT, P, TOK], F32)
    xf_in = nc.dram_tensor("xf_in", [CT, P, TOK], F32)
    xf_out = nc.dram_tensor("xf_out", [NC_, CT, P, TOK], F32, addr_space="Shared")

    RG = [list(range(NC_))]

    ctxs = []

    tc = tile.TileContext(nc)
    tc.__enter__()
    try:
        def pool(name, bufs, space="SBUF"):
            p_ = tc.tile_pool(name=name, bufs=bufs, space=space)
            ctxs.append(p_)
            return p_.__enter__()

        cpool = pool("const", 1)
        xpool = pool("xp", 1)
        wkp = pool("wk", 1)
        hfp = pool("hf", 1)
        h1p = pool("h1", 1)
        wst = pool("wst", 2)
        stg = pool("stg", 2)
        psp = pool("ps", 5, space="PSUM")
        pss = pool("pss", 1, space="PSUM")

        ident = cpool.tile([P, P], F32)
        make_identity(nc, ident[:])
        ones_col = cpool.tile([P, 1], F32R)
        nc.sync.dma_start(out=ones_col[:], in_=onescol.ap())
        ones_row_r = cpool.tile([1, P], F32R)
        nc.sync.dma_start(out=ones_row_r[:], in_=onesrow.ap())
        # expert-row selectors: sel[:, e, :] is a [E, P] lhsT picking G row e
        sel8 = cpool.tile([E, E, P], F32R)
        nc.sync.dma_start(out=sel8[:], in_=sel8c.ap())
        ind8 = cpool.tile([E, TR_MT, P], F32R)
        nc.sync.dma_start(out=ind8[:], in_=ind8c.ap())
        sel16 = cpool.tile([E, E, P], BF16)
        nc.sync.dma_start(out=sel16[:], in_=sel8b.ap())
        ind16 = cpool.tile([E, TR_MT, P], BF16)
        nc.sync.dma_start(out=ind16[:], in_=ind8b.ap())

        def c32(ap):
            return ap.bitcast(F32)

        x = xpool.tile([P, CT, TOK], F32R, tag="x")
        nc.sync.dma_start(out=x[:], in_=x0T.ap().rearrange("kt p t -> p kt t"))
        vf = xpool.tile([P, CT, TOK], F32R, tag="vf")

        def layer_norm(src, s_ap, b_ap):
            """src [P, CT, TOK] f32r -> normalized f32r tile (tag lnout)."""
            x2 = wkp.tile([P, CT, TOK], F32R, tag="lnx2")
            nc.vector.tensor_tensor(out=x2[:], in0=src[:], in1=src[:], op=OP.mult)
            ps_st = pss.tile([1, 2, TOK], F32, space="PSUM", tag="lnstats")
            for k in range(CT):
                nc.tensor.matmul(ps_st[:, 0, :], c32(ones_col[:]), c32(src[:, k, :]),
                                 start=(k == 0), stop=(k == CT - 1))
            for k in range(CT):
                nc.tensor.matmul(ps_st[:, 1, :], c32(ones_col[:]), c32(x2[:, k, :]),
                                 start=(k == 0), stop=(k == CT - 1))
            stats = wkp.tile([1, 4, TOK], F32, tag="lnsts")
            mean, ex2, m2, var = (stats[:, i, :] for i in range(4))
            nc.vector.tensor_scalar(out=mean, in0=ps_st[:, 0, :], scalar1=1.0 / C,
                                    scalar2=None, op0=OP.mult)
            nc.vector.tensor_scalar(out=ex2, in0=ps_st[:, 1, :], scalar1=1.0 / C,
                                    scalar2=None, op0=OP.mult)
            nc.vector.tensor_tensor(out=m2, in0=mean, in1=mean, op=OP.mult)
            nc.vector.tensor_tensor(out=var, in0=ex2, in1=m2, op=OP.subtract)
            nc.vector.tensor_scalar(out=var, in0=var, scalar1=EPS, scalar2=None,
                                    op0=OP.add)
            nr = wkp.tile([1, 2, TOK], F32R, tag="lnnr")
            rstd_f = wkp.tile([1, TOK], F32, tag="lnrstd")
            nc.scalar.activation(rstd_f[:], var, AF.Sqrt)
            nc.vector.reciprocal(rstd_f[:], rstd_f[:])
            nc.vector.tensor_copy(nr[:, 0, :], rstd_f[:])
            nc.vector.tensor_scalar(out=nr[:, 1, :], in0=mean, scalar1=-1.0,
                                    scalar2=None, op0=OP.mult)
            ps_b = pss.tile([P, 2, TOK], F32, space="PSUM", tag="lnb")
            nc.tensor.matmul(ps_b[:, 0, :], c32(ones_row_r[:]), c32(nr[:, 0, :]),
                             start=True, stop=True)
            nc.tensor.matmul(ps_b[:, 1, :], c32(ones_row_r[:]), c32(nr[:, 1, :]),
                             start=True, stop=True)
            out = wkp.tile([P, CT, TOK], F32R, tag="lnout")
            for k in range(CT):
                nc.vector.tensor_tensor(out=out[:, k, :], in0=src[:, k, :],
                                        in1=ps_b[:, 1, :], op=OP.add)
                nc.vector.tensor_tensor(out=out[:, k, :], in0=out[:, k, :],
                                        in1=ps_b[:, 0, :], op=OP.mult)
            st = wkp.tile([P, 2, CT], F32, tag="lnsc")
            nc.sync.dma_start(out=st[:, 0, :], in_=s_ap)
            nc.sync.dma_start(out=st[:, 1, :], in_=b_ap)
            for k in range(CT):
                nc.vector.tensor_scalar(out=out[:, k, :], in0=out[:, k, :],
                                        scalar1=st[:, 0, k:k + 1],
                                        scalar2=st[:, 1, k:k + 1],
                                        op0=OP.mult, op1=OP.add)
            return out

        def proj(w_dram_l, rhs, out, act=None, accum_into=None):
            for m in range(CT):
                w_t = wst.tile([P, CT, P], F32R, tag="wst6")
                nc.sync.dma_start(
                    out=w_t[:],
                    in_=w_dram_l[:, m * P:(m + 1) * P].rearrange(
                        "(kt p) n -> p kt n", p=P))
                ps = psp.tile([P, CH], F32, space="PSUM", tag="mm")
                for k in range(CT):
                    nc.tensor.matmul(ps[:, 0:TOK], c32(w_t[:, k, :]),
                                     c32(rhs[:, k, :]),
                                     start=(k == 0), stop=(k == CT - 1))
                if accum_into is not None:
                    nc.vector.tensor_tensor(out=accum_into[:, m, :],
                                            in0=accum_into[:, m, :],
                                            in1=ps[:, 0:TOK], op=OP.add)
                elif act is not None:
                    nc.scalar.activation(out[:, m, :], ps[:, 0:TOK], act)
                else:
                    nc.vector.tensor_copy(out[:, m, :], ps[:, 0:TOK])

        for l in range(L):
            xln = layer_norm(x, ln1s.ap()[l], ln1b.ap()[l])

            sigr = wkp.tile([P, CT, TOK], F32, tag="sigr")
            kk = wkp.tile([P, CT, TOK], F32, tag="kk")
            vv = wkp.tile([P, CT, TOK], F32, tag="vv")
            proj(wr.ap()[l], xln, sigr, act=AF.Sigmoid)
            proj(wk.ap()[l], xln, kk)
            proj(wv.ap()[l], xln, vv)
            if l == 0:
                nc.vector.tensor_copy(vf[:], vv[:])
            else:
                nc.vector.tensor_tensor(out=vv[:], in0=vv[:], in1=vf[:], op=OP.add)
                nc.vector.tensor_scalar(out=vv[:], in0=vv[:], scalar1=0.5,
                                        scalar2=None, op0=OP.mult)
            kv = wkp.tile([P, CT, TOK], F32, tag="kv")
            nc.vector.tensor_tensor(out=kv[:], in0=kk[:], in1=vv[:], op=OP.mult)

            dtile = wkp.tile([P, CT], F32, tag="dt")
            nc.sync.dma_start(out=dtile[:], in_=dsig.ap()[l])
            states = wkp.tile([P, CT, TOK], F32R, tag="states")
            for k in range(CT):
                nc.vector.tensor_tensor_scan(
                    states[:, k, :], dtile[:, k:k + 1].to_broadcast([P, TOK]),
                    kv[:, k, :], 0.0, op0=OP.mult, op1=OP.add)
            fin = wkp.tile([P, CT], F32, tag="fin")
            for k in range(CT):
                nc.vector.tensor_copy(fin[:, k:k + 1], states[:, k, TOK - 1:TOK])
            nc.sync.dma_start(out=fin_in.ap(), in_=fin[:])
            nc.gpsimd.collective_compute(
                "AllGather", OP.bypass, replica_groups=RG,
                ins=[fin_in.ap().opt()], outs=[fin_out.ap().opt()])
            lt = wkp.tile([P, CT, NC_], F32, tag="lfin")
            nc.sync.dma_start(out=lt[:],
                              in_=fin_out.ap().rearrange("(m p) kt -> p kt m", p=P))
            pt = wkp.tile([P, CT, NC_], F32, tag="pfin")
            nc.sync.dma_start(out=pt[:], in_=scanP.ap()[l])
            nc.vector.tensor_tensor(out=lt[:], in0=lt[:], in1=pt[:], op=OP.mult)
            init = wkp.tile([P, CT], F32, tag="init")
            nc.vector.tensor_reduce(init[:], lt[:], axis=AX.X, op=OP.add)
            for k in range(CT):
                nc.vector.tensor_tensor_scan(
                    states[:, k, :], dtile[:, k:k + 1].to_broadcast([P, TOK]),
                    kv[:, k, :], init[:, k:k + 1], op0=OP.mult, op1=OP.add)
            satt = wkp.tile([P, CT, TOK], F32R, tag="lnx2")
            nc.vector.tensor_tensor(out=satt[:], in0=sigr[:], in1=states[:],
                                    op=OP.mult)
            proj(wo.ap()[l], satt, None, accum_into=x)

            h = layer_norm(x, ln2s.ap()[l], ln2b.ap()[l])

            # router
            rwt = wkp.tile([P, CT, 16], F32R, tag="rwt")
            nc.sync.dma_start(out=rwt[:],
                              in_=routerW.ap()[l].rearrange("(kt p) n -> p kt n", p=P))
            se_t = wkp.tile([P, 2, E], F32, tag="sht")
            nc.sync.dma_start(out=se_t[:, 0, :], in_=sharesb.ap()[l])
            nc.sync.dma_start(out=se_t[:, 1, :], in_=confb.ap()[l])
            bids_sb = wkp.tile([P, TOK // P, E], F32, tag="bids")
            for m in range(TOK // P):
                ps = psp.tile([P, CH], F32, space="PSUM", tag="mm")
                for k in range(CT):
                    nc.tensor.matmul(ps[:, 0:16], c32(h[:, k, m * P:(m + 1) * P]),
                                     c32(rwt[:, k, :]), start=(k == 0),
                                     stop=(k == CT - 1))
                tmp = wkp.tile([P, E], F32, tag="rtmp")
                nc.vector.tensor_tensor(out=tmp[:], in0=ps[:, 0:E],
                                        in1=se_t[:, 1, :], op=OP.add)
                nc.scalar.activation(tmp[:], tmp[:], AF.Sigmoid)
                nc.vector.tensor_tensor(out=tmp[:], in0=tmp[:], in1=se_t[:, 0, :],
                                        op=OP.mult)
                nc.vector.tensor_tensor(out=bids_sb[:, m, :], in0=tmp[:],
                                        in1=ps[:, E:16], op=OP.add)
            nc.sync.dma_start(out=bids_in.ap().rearrange("(m p) e -> p m e", p=P),
                              in_=bids_sb[:])
            nc.gpsimd.collective_compute(
                "AllGather", OP.bypass, replica_groups=RG,
                ins=[bids_in.ap().opt()], outs=[bids_out.ap().opt()])

            # bridge -> prefix
            prefix = wkp.tile([P, CT, TOK], F32R, tag="prefix")
            if l == 1:
                hs16 = wkp.tile([P, 2 * CT, TOK], BF16, tag="hs16")
                nc.vector.tensor_copy(hs16[:, 0:CT, :], h[:])
                nc.vector.tensor_copy(hs16[:, CT:2 * CT, :], states[:])
            for m in range(CT):
                if l == 0:
                    w_t = wst.tile([P, 2 * CT, P], F32R, tag="wst12")
                    nc.sync.dma_start(
                        out=w_t[:],
                        in_=bridgeW.ap()[:, m * P:(m + 1) * P].rearrange(
                            "(kt p) n -> p kt n", p=P))
                else:
                    w_t = wst.tile([P, 2 * CT, P], BF16, tag="wsb12")
                    nc.sync.dma_start(
                        out=w_t[:],
                        in_=bridgeb.ap()[:, m * P:(m + 1) * P].rearrange(
                            "(kt p) n -> p kt n", p=P))
                ps = psp.tile([P, CH], F32, space="PSUM", tag="mm")
                for k in range(2 * CT):
                    if l == 0:
                        rhs = h[:, k, :] if k < CT else states[:, k - CT, :]
                        nc.tensor.matmul(ps[:, 0:TOK], c32(w_t[:, k, :]), c32(rhs),
                                         start=(k == 0), stop=(k == 2 * CT - 1))
                    else:
                        nc.tensor.matmul(ps[:, 0:TOK], w_t[:, k, :], hs16[:, k, :],
                                         start=(k == 0), stop=(k == 2 * CT - 1))
                nc.scalar.activation(prefix[:, m, :], ps[:, 0:TOK], AF.Tanh)

            nc.sync.dma_start(out=hp_in.ap()[0:CT].rearrange("kt p t -> p kt t"),
                              in_=h[:])
            nc.sync.dma_start(
                out=hp_in.ap()[CT:2 * CT].rearrange("kt p t -> p kt t"),
                in_=prefix[:])
            nc.gpsimd.collective_compute(
                "AllGather", OP.bypass, replica_groups=RG,
                ins=[hp_in.ap().opt()], outs=[hp_out.ap().opt()])

            # gates from gathered bids: G [E, 16, P], token t = g*128 + p
            bt = wkp.tile([P, 16, E], F32, tag="btile")
            nc.sync.dma_start(out=bt[:],
                              in_=bids_out.ap().rearrange("(g p) e -> p g e", p=P))
            m1 = wkp.tile([P, 2, 16], F32, tag="m1")
            nc.vector.tensor_reduce(m1[:, 0, :], bt[:], axis=AX.X, op=OP.max)
            eq1 = wkp.tile([P, 16, E], F32, tag="eq1")
            nc.vector.tensor_tensor(out=eq1[:], in0=bt[:],
                                    in1=m1[:, 0, :].to_broadcast([P, 16, E]),
                                    op=OP.is_equal)
            msk = wkp.tile([P, 16, E], F32, tag="msk")
            nc.vector.scalar_tensor_tensor(out=msk[:], in0=eq1[:], scalar=-1e30,
                                           in1=bt[:], op0=OP.mult, op1=OP.add)
            nc.vector.tensor_reduce(m1[:, 1, :], msk[:], axis=AX.X, op=OP.max)
            eq2 = wkp.tile([P, 16, E], F32, tag="eq2")
            nc.vector.tensor_tensor(out=eq2[:], in0=msk[:],
                                    in1=m1[:, 1, :].to_broadcast([P, 16, E]),
                                    op=OP.is_equal)
            wg = wkp.tile([P, 2, 16], F32, tag="wg")
            nc.vector.tensor_tensor(out=wg[:, 1, :], in0=m1[:, 1, :],
                                    in1=m1[:, 0, :], op=OP.subtract)
            nc.scalar.activation(wg[:, 1, :], wg[:, 1, :], AF.Sigmoid)
            nc.vector.tensor_scalar(out=wg[:, 0, :], in0=wg[:, 1, :], scalar1=-1.0,
                                    scalar2=1.0, op0=OP.mult, op1=OP.add)
            gt = wkp.tile([P, 16, E], F32, tag="gt")
            nc.vector.tensor_tensor(out=gt[:], in0=eq1[:],
                                    in1=wg[:, 0, :].to_broadcast([P, 16, E]),
                                    op=OP.mult)
            g2t = wkp.tile([P, 16, E], F32, tag="g2t")
            nc.vector.tensor_tensor(out=g2t[:], in0=eq2[:],
                                    in1=wg[:, 1, :].to_broadcast([P, 16, E]),
                                    op=OP.mult)
            nc.vector.tensor_tensor(out=gt[:], in0=gt[:], in1=g2t[:], op=OP.add)
            G = wkp.tile([E, 16, P], F32R, tag="G")
            G16 = wkp.tile([E, 16, P], BF16, tag="G16")
            for g in range(16):
                psg = pss.tile([E, P], F32, space="PSUM", tag="psg")
                nc.tensor.transpose(psg[:], gt[:, g, :], ident[:])
                if l == 0:
                    nc.vector.tensor_copy(G[:, g, :], psg[:])
                else:
                    nc.vector.tensor_copy(G16[:, g, :], psg[:])

            # ---- expert phase (dense, H-sharded); chunk = one rank's tokens
            for c in range(NC_):
                DTE = F32R if l == 0 else BF16
                h_chf = h1p.tile([P, CT, ECH], F32R, tag="hch")
                pf_chf = h1p.tile([P, CT, ECH], F32R, tag="pfch")
                nc.sync.dma_start(
                    out=h_chf[:],
                    in_=hp_out.ap()[c, 0:CT].rearrange("kt p t -> p kt t"))
                nc.sync.dma_start(
                    out=pf_chf[:],
                    in_=hp_out.ap()[c, CT:2 * CT].rearrange("kt p t -> p kt t"))
                if l == 0:
                    h_ch, pf_ch = h_chf, pf_chf
                else:
                    h_ch = h1p.tile([P, CT, ECH], BF16, tag="hch16")
                    pf_ch = h1p.tile([P, CT, ECH], BF16, tag="pfch16")
                    nc.vector.tensor_copy(h_ch[:], h_chf[:])
                    nc.vector.tensor_copy(pf_ch[:], pf_chf[:])
                h1g = h1p.tile([P, RWKV_MT + TR_MT, ECH], DTE, tag="h1g")
                for mt in range(RWKV_MT):
                    e = mt // (HS // P)
                    if l == 0:
                        w_t = wst.tile([P, CT, P], F32R, tag="wst6")
                        nc.sync.dma_start(
                            out=w_t[:],
                            in_=w1s.ap()[l][:, mt * P:(mt + 1) * P].rearrange(
                                "(kt p) n -> p kt n", p=P))
                    else:
                        w_t = wst.tile([P, CT, P], BF16, tag="wsb6")
                        nc.sync.dma_start(
                            out=w_t[:],
                            in_=w1b.ap()[:, mt * P:(mt + 1) * P].rearrange(
                                "(kt p) n -> p kt n", p=P))
                    ps = psp.tile([P, CH], F32, space="PSUM", tag="mm")
                    for k in range(CT):
                        if l == 0:
                            nc.tensor.matmul(ps[:, 0:ECH], c32(w_t[:, k, :]),
                                             c32(h_ch[:, k, :]),
                                             start=(k == 0), stop=(k == CT - 1))
                        else:
                            nc.tensor.matmul(ps[:, 0:ECH], w_t[:, k, :],
                                             h_ch[:, k, :],
                                             start=(k == 0), stop=(k == CT - 1))
                    rl = stg.tile([P, ECH], F32, tag="rl")
                    nc.scalar.activation(rl[:], ps[:, 0:ECH], AF.Relu)
                    nc.scalar.activation(h1g[:, mt, :], rl[:], AF.Square)
                    gps = psp.tile([P, CH], F32, space="PSUM", tag="mm")
                    Gs = G if l == 0 else G16
                    gsl = Gs[:, 2 * c:2 * (c + 1), :].rearrange("a g p -> a (g p)")
                    if l == 0:
                        nc.tensor.matmul(gps[:, 0:ECH], c32(sel8[:, e, :]), c32(gsl),
                                         start=True, stop=True)
                    else:
                        nc.tensor.matmul(gps[:, 0:ECH], sel16[:, e, :], gsl,
                                         start=True, stop=True)
                    nc.vector.tensor_tensor(out=h1g[:, mt, :], in0=h1g[:, mt, :],
                                            in1=gps[:, 0:ECH], op=OP.mult)
                # trans experts
                pz_t = h1p.tile([P, TR_MT, ECH], F32, tag="pz_t")
                for mt in range(TR_MT):
                    if l == 0:
                        w_t = wst.tile([P, CT, P], F32R, tag="wst6")
                        nc.sync.dma_start(
                            out=w_t[:],
                            in_=aps_.ap()[l][:, (TR_MT + mt) * P:(TR_MT + mt + 1) * P]
                            .rearrange("(kt p) n -> p kt n", p=P))
                    else:
                        w_t = wst.tile([P, CT, P], BF16, tag="wsb6")
                        nc.sync.dma_start(
                            out=w_t[:],
                            in_=apb.ap()[:, (TR_MT + mt) * P:(TR_MT + mt + 1) * P]
                            .rearrange("(kt p) n -> p kt n", p=P))
                    ps = psp.tile([P, CH], F32, space="PSUM", tag="mm")
                    for k in range(CT):
                        if l == 0:
                            nc.tensor.matmul(ps[:, 0:ECH], c32(w_t[:, k, :]),
                                             c32(pf_ch[:, k, :]),
                                             start=(k == 0), stop=(k == CT - 1))
                        else:
                            nc.tensor.matmul(ps[:, 0:ECH], w_t[:, k, :],
                                             pf_ch[:, k, :],
                                             start=(k == 0), stop=(k == CT - 1))
                    nc.vector.tensor_copy(pz_t[:, mt, :], ps[:, 0:ECH])
                for mt in range(TR_MT):
                    if l == 0:
                        w_t = wst.tile([P, CT, P], F32R, tag="wst6")
                        nc.sync.dma_start(
                            out=w_t[:],
                            in_=aps_.ap()[l][:, mt * P:(mt + 1) * P].rearrange(
                                "(kt p) n -> p kt n", p=P))
                    else:
                        w_t = wst.tile([P, CT, P], BF16, tag="wsb6")
                        nc.sync.dma_start(
                            out=w_t[:],
                            in_=apb.ap()[:, mt * P:(mt + 1) * P].rearrange(
                                "(kt p) n -> p kt n", p=P))
                    ps = psp.tile([P, CH], F32, space="PSUM", tag="mm")
                    for k in range(CT):
                        if l == 0:
                            nc.tensor.matmul(ps[:, 0:ECH], c32(w_t[:, k, :]),
                                             c32(h_ch[:, k, :]),
                                             start=(k == 0), stop=(k == CT - 1))
                        else:
                            nc.tensor.matmul(ps[:, 0:ECH], w_t[:, k, :],
                                             h_ch[:, k, :],
                                             start=(k == 0), stop=(k == CT - 1))
                    aidx = RWKV_MT + mt
                    nc.scalar.activation(h1g[:, aidx, :], ps[:, 0:ECH], AF.Silu)
                    nc.vector.tensor_tensor(out=h1g[:, aidx, :],
                                            in0=h1g[:, aidx, :],
                                            in1=pz_t[:, mt, :], op=OP.mult)
                    gps = psp.tile([P, CH], F32, space="PSUM", tag="mm")
                    Gs = G if l == 0 else G16
                    gsl = Gs[:, 2 * c:2 * (c + 1), :].rearrange("a g p -> a (g p)")
                    if l == 0:
                        nc.tensor.matmul(gps[:, 0:ECH], c32(ind8[:, mt, :]), c32(gsl),
                                         start=True, stop=True)
                    else:
                        nc.tensor.matmul(gps[:, 0:ECH], ind16[:, mt, :], gsl,
                                         start=True, stop=True)
                    nc.vector.tensor_tensor(out=h1g[:, aidx, :],
                                            in0=h1g[:, aidx, :],
                                            in1=gps[:, 0:ECH], op=OP.mult)
                # second matmuls accumulate all experts
                for m in range(CT):
                    ps = psp.tile([P, CH], F32, space="PSUM", tag="mm")
                    for e in range(E_R):
                        if l == 0:
                            w_t = wst.tile([P, HS // P, P], F32R, tag="wst3")
                            nc.sync.dma_start(
                                out=w_t[:],
                                in_=w2s.ap()[l][e * HS:(e + 1) * HS,
                                                m * P:(m + 1) * P]
                                .rearrange("(kt p) n -> p kt n", p=P))
                        else:
                            w_t = wst.tile([P, HS // P, P], BF16, tag="wsb3")
                            nc.sync.dma_start(
                                out=w_t[:],
                                in_=w2b.ap()[e * HS:(e + 1) * HS, m * P:(m + 1) * P]
                                .rearrange("(kt p) n -> p kt n", p=P))
                        for k2 in range(HS // P):
                            if l == 0:
                                nc.tensor.matmul(ps[:, 0:ECH], c32(w_t[:, k2, :]),
                                                 c32(h1g[:, e * (HS // P) + k2, :]),
                                                 start=(e == 0 and k2 == 0),
                                                 stop=False)
                            else:
                                nc.tensor.matmul(ps[:, 0:ECH], w_t[:, k2, :],
                                                 h1g[:, e * (HS // P) + k2, :],
                                                 start=(e == 0 and k2 == 0),
                                                 stop=False)
                    if l == 0:
                        w_t = wst.tile([P, TR_MT, P], F32R, tag="wst3")
                        nc.sync.dma_start(
                            out=w_t[:],
                            in_=bs_.ap()[l][:, m * P:(m + 1) * P].rearrange(
                                "(kt p) n -> p kt n", p=P))
                    else:
                        w_t = wst.tile([P, TR_MT, P], BF16, tag="wsb3")
                        nc.sync.dma_start(
                            out=w_t[:],
                            in_=bsb.ap()[:, m * P:(m + 1) * P].rearrange(
                                "(kt p) n -> p kt n", p=P))
                    for k2 in range(TR_MT):
                        if l == 0:
                            nc.tensor.matmul(ps[:, 0:ECH], c32(w_t[:, k2, :]),
                                             c32(h1g[:, RWKV_MT + k2, :]),
                                             start=False, stop=(k2 == TR_MT - 1))
                        else:
                            nc.tensor.matmul(ps[:, 0:ECH], w_t[:, k2, :],
                                             h1g[:, RWKV_MT + k2, :],
                                             start=False, stop=(k2 == TR_MT - 1))
                    st = stg.tile([P, ECH], F32, tag="w2out")
                    if m % 2 == 0:
                        nc.vector.tensor_copy(st[:], ps[:, 0:ECH])
                    else:
                        nc.scalar.activation(st[:], ps[:, 0:ECH], AF.Copy)
                    nc.sync.dma_start(out=rs_in.ap()[c, m], in_=st[:])
            nc.gpsimd.collective_compute(
                "ReduceScatter", OP.add, replica_groups=RG,
                ins=[rs_in.ap().opt()], outs=[rs_out.ap().opt()])
            moe = wkp.tile([P, CT, TOK], F32, tag="kv")
            nc.sync.dma_start(out=moe[:],
                              in_=rs_out.ap().rearrange("kt p t -> p kt t"))
            nc.vector.tensor_tensor(out=x[:], in0=x[:], in1=moe[:], op=OP.add)

        # final layernorm + allgather + head
        xf = layer_norm(x, lnos.ap(), lnob.ap())
        xfc = wkp.tile([P, CT, TOK], F32, tag="kv")
        nc.vector.tensor_copy(xfc[:], xf[:])
        nc.sync.dma_start(out=xf_in.ap().rearrange("kt p t -> p kt t"), in_=xfc[:])
        nc.gpsimd.collective_compute(
            "AllGather", OP.bypass, replica_groups=RG,
            ins=[xf_in.ap().opt()], outs=[xf_out.ap().opt()])
        xf_full = hfp.tile([P, CT, M], BF16, tag="xffull")
        for r in range(NC_):
            nc.gpsimd.dma_start(
                out=xf_full[:, :, r * TOK:(r + 1) * TOK],
                in_=xf_out.ap()[r].rearrange("kt p t -> p kt t"))
        for m in range(HEAD_MT):
            w_t = wst.tile([P, CT, P], BF16, tag="whead")
            nc.sync.dma_start(
                out=w_t[:],
                in_=headw.ap()[:, m * P:(m + 1) * P].rearrange(
                    "(kt p) n -> p kt n", p=P))
            for c4 in range(NCH):
                ps = psp.tile([P, CH], F32, space="PSUM", tag="mm")
                for k in range(CT):
                    nc.tensor.matmul(ps[:], w_t[:, k, :],
                                     xf_full[:, k, c4 * CH:(c4 + 1) * CH],
                                     start=(k == 0), stop=(k == CT - 1))
                st = stg.tile([P, CH], BF16, tag="hout")
                if c4 % 2 == 0:
                    nc.vector.tensor_copy(st[:], ps[:])
                else:
                    nc.scalar.activation(st[:], ps[:], AF.Copy)
                nc.sync.dma_start(
                    out=logitsT.ap()[m * P:(m + 1) * P, c4 * CH:(c4 + 1) * CH],
                    in_=st[:])
    finally:
        for p_ in reversed(ctxs):
            p_.__exit__(None, None, None)
        tc.__exit__(None, None, None)

    nc.compile()
    return nc


def _sel8_const():
    s = np.zeros((E, E, P), np.float32)
    for e in range(E):
        s[e, e, :] = 1.0
    return s


def _ind8_const():
    s = np.zeros((E, TR_MT, P), np.float32)
    s[E_R, 0, :] = 1.0
    s[E_R, 1, 0:64] = 1.0
    s[E_R + 1, 1, 64:128] = 1.0
    s[E_R + 1, 2, :] = 1.0
    return s


def _host_prep(inputs):
    f32 = np.float32
    idx = np.asarray(inputs["idx"]).astype(np.int64)
    emb_W = np.asarray(inputs["emb_W"], dtype=f32)
    x0 = emb_W[idx.reshape(-1)]                      # [M, C]
    decay = np.asarray(inputs["decay"], dtype=f32)
    d = (1.0 / (1.0 + np.exp(-decay.astype(np.float64)))).astype(np.float64)  # [L,C]
    caps = np.asarray(inputs["capital_shares"], dtype=f32)
    shares = caps / caps.sum(axis=1, keepdims=True)  # [L, E]

    def chanlay(a):
        a = np.asarray(a, dtype=f32)
        return np.ascontiguousarray(a.reshape(*a.shape[:-1], CT, P).swapaxes(-1, -2))

    conf_w = np.asarray(inputs["conf_w"], dtype=f32)
    critic = np.asarray(inputs["critic_Wa"], dtype=f32)
    routerW = np.ascontiguousarray(
        np.concatenate([conf_w.transpose(0, 2, 1), critic], axis=2))

    ffn_W1 = np.asarray(inputs["ffn_W1"], dtype=f32)
    ffn_W2 = np.asarray(inputs["ffn_W2"], dtype=f32)
    tA = np.asarray(inputs["trans_A"], dtype=f32)
    tP = np.asarray(inputs["trans_P"], dtype=f32)
    tB = np.asarray(inputs["trans_B"], dtype=f32)
    head_W = np.asarray(inputs["head_W"], dtype=f32)
    conf_b = np.asarray(inputs["conf_b"], dtype=f32)  # [L, E]

    shared = dict(
        wr=np.ascontiguousarray(inputs["Wr"], dtype=f32),
        wk=np.ascontiguousarray(inputs["Wk"], dtype=f32),
        wv=np.ascontiguousarray(inputs["Wv"], dtype=f32),
        wo=np.ascontiguousarray(inputs["Wo"], dtype=f32),
        dsig=chanlay(d.astype(f32)),
        ln1s=chanlay(inputs["ln1_s"]), ln1b=chanlay(inputs["ln1_b"]),
        ln2s=chanlay(inputs["ln2_s"]), ln2b=chanlay(inputs["ln2_b"]),
        lnos=chanlay(inputs["lnout_s"]), lnob=chanlay(inputs["lnout_b"]),
        routerW=routerW,
        confb=np.ascontiguousarray(
            np.broadcast_to(conf_b[:, None, :], (L, P, E)).astype(f32)),
        sharesb=np.ascontiguousarray(
            np.broadcast_to(shares[:, None, :], (L, P, E)).astype(f32)),
        bridgeW=np.ascontiguousarray(inputs["bridge_W"], dtype=f32),
        sel8c=_sel8_const(), ind8c=_ind8_const(),
        sel8b=_sel8_const().astype(ml_dtypes.bfloat16),
        ind8b=_ind8_const().astype(ml_dtypes.bfloat16),
        bridgeb=np.ascontiguousarray(
            np.asarray(inputs["bridge_W"]).astype(ml_dtypes.bfloat16)),
        onescol=np.ones((P, 1), np.float32),
        onesrow=np.ones((1, P), np.float32),
    )

    in_maps = []
    for i in range(NC_):
        b_idx, j = divmod(i, NC_ // B)
        scanP_l = np.zeros((L, C, NC_), np.float64)
        for ll in range(L):
            for mprev in range(j):
                ridx = b_idx * (NC_ // B) + mprev
                scanP_l[ll, :, ridx] = d[ll] ** (256.0 * (j - mprev - 1))
        scanP_lay = np.ascontiguousarray(
            scanP_l.astype(f32).reshape(L, CT, P, NC_).swapaxes(1, 2))

        w1c = ffn_W1[:, :, :, i * HS:(i + 1) * HS]
        w1c = np.ascontiguousarray(w1c.transpose(0, 2, 1, 3).reshape(L, C, E_R * HS))
        w2c = np.ascontiguousarray(
            ffn_W2[:, :, i * HS:(i + 1) * HS, :].reshape(L, E_R * HS, C))
        a_s = tA[:, :, :, i * H2S:(i + 1) * H2S].transpose(0, 2, 1, 3)
        a_s = a_s.reshape(L, C, E_T * H2S)
        p_s = tP[:, :, :, i * H2S:(i + 1) * H2S].transpose(0, 2, 1, 3)
        p_s = p_s.reshape(L, C, E_T * H2S)
        apsc = np.ascontiguousarray(np.concatenate([a_s, p_s], axis=2))
        b_c = np.ascontiguousarray(
            tB[:, :, i * H2S:(i + 1) * H2S, :].reshape(L, E_T * H2S, C))

        hw = np.zeros((C, VSP), f32)
        lo = i * VS
        hi = min((i + 1) * VS, V)
        hw[:, :hi - lo] = head_W[:, lo:hi]

        x0T = np.ascontiguousarray(
            x0[i * TOK:(i + 1) * TOK].T.reshape(CT, P, TOK))

        im = dict(shared)
        im.update(
            x0T=x0T.astype(f32),
            scanP=scanP_lay,
            w1s=w1c, w2s=w2c, aps=apsc, bs=b_c,
            w1b=np.ascontiguousarray(w1c[1].astype(ml_dtypes.bfloat16)),
            w2b=np.ascontiguousarray(w2c[1].astype(ml_dtypes.bfloat16)),
            apb=np.ascontiguousarray(apsc[1].astype(ml_dtypes.bfloat16)),
            bsb=np.ascontiguousarray(b_c[1].astype(ml_dtypes.bfloat16)),
            headw=np.ascontiguousarray(hw.astype(ml_dtypes.bfloat16)),
        )
        in_maps.append(im)
    return in_maps


def kernel(**inputs):
    if "nc" not in _CACHE:
        _CACHE["nc"] = _build()
    nc = _CACHE["nc"]
    in_maps = _host_prep(inputs)
    trace = os.environ.get("K_TRACE", "0") == "1"
    res = bass_utils.run_bass_kernel_spmd(nc, in_maps, core_ids=list(range(NC_)),
                                          trace=trace)
    _CACHE["last_res"] = res
    outs = []
    for i in range(NC_):
        lt = np.asarray(res.results[i]["logitsT"], dtype=np.float32)
        lo = i * VS
        hi = min((i + 1) * VS, V)
        outs.append(lt[: hi - lo].T)
    full = np.concatenate(outs, axis=1)
    return full.reshape(B, T, V).astype(np.float32)

